# revision 37
# baseline (speedup 1.0000x reference)
"""Trainium2 Bass kernel for nn_Attention_76802605187548.

Attention-OCR model: 7-layer CNN encoder (3 maxpools, 3 train-mode BNs) +
49-step attention-LSTM decoder.

Sharding: pure data parallelism — batch 16 -> 8 cores x 2 images. All params
replicated. BN runs in training mode with FULL-batch statistics, so each BN
does one tiny AllReduce of per-channel (sum, sumsq) across the 8 cores
(local 2-image stats would give ~4% output error — measured).

Compute: all matmuls in float32r (1 cycle/row on the PE at N>=256,
~1.5e-4 relative error per matmul vs 4 cycles/row for plain fp32).
Convs are emitted as 9 (or 4) shifted-window matmuls accumulating in PSUM;
feature maps stay SBUF-resident end-to-end (no DRAM spills). Maxpools are
fused into PSUM evacuation (DVE relu+pair-max, GPSIMD x-max). The decoder
keeps H=128 on partitions, folds the attention context directly into the
LSTM gate matmul via a precomputed (Wih_ctx @ src) tensor, and runs the
softmax un-normalized with the 1/sum folded into alpha.
"""
import os
import numpy as np
from contextlib import ExitStack

import concourse.bass as bass
import concourse.bacc as bacc
import concourse.tile as tile
import concourse.mybir as mybir
from concourse.bass_utils import run_bass_kernel_spmd
from concourse.masks import make_identity

F32 = mybir.dt.float32
R = mybir.dt.float32r
I32 = mybir.dt.int32
AF = mybir.ActivationFunctionType
OP = mybir.AluOpType

N_CORES = 8
BPC = 2                     # images per core
T_STEPS = 49
L = 903                     # 7*129 attention positions
HID = 128
SRC = 512
NCLS = 245
VOCAB = 246

_uid = [0]


def _nm(p):
    _uid[0] += 1
    return f"{p}{_uid[0]}"


# ------------------------------------------------------------------ host prep
def _host_prep(image, text, conv_params, bn_params, dec_params):
    """Pure layout transforms (transpose / reshape / pad / index packing)."""
    cp = [np.asarray(x, np.float32) for x in conv_params]
    g2, bt2, g4, bt4, g6, bt6 = [np.asarray(x, np.float32) for x in bn_params]
    (emb_tab, Wi, Wh, bh, ws, Wih, bih, Whh, bhh, Wg, bg) = [
        np.asarray(x, np.float32) for x in dec_params]
    ws_ = {}

    def wblocks(w):
        # w [Co, Ci, kh, kw] -> [mt*kin*taps, 128, cow] blocks of w.T per
        # (m, k, tap); zero-padded in the Ci direction.
        Co, Ci, kh, kw = w.shape
        mt, kin = (Co + 127) // 128, (Ci + 127) // 128
        cow = min(128, Co)
        out = np.zeros((mt * kin * kh * kw, 128, cow), np.float32)
        i = 0
        for m in range(mt):
            for k in range(kin):
                for dy in range(kh):
                    for dx in range(kw):
                        blk = w[m * 128:(m + 1) * 128,
                                k * 128:(k + 1) * 128, dy, dx].T  # [ci, co]
                        out[i, :blk.shape[0], :blk.shape[1]] = blk
                        i += 1
        return out

    # L0: [128, 128]; strips g=2b+h at partitions 32g; b0 strips write
    # output cols 0:64, b1 strips cols 64:128 (block-diagonal so one K=74
    # matmul computes both images; zero cols mask the in-between partitions)
    w0, b0 = cp[0], cp[1]
    # strip g = 2*h + b at partitions 32g; even strips (b=0) -> out cols
    # 0:64, odd strips (b=1) -> cols 64:128; matmul h spans [64h, 64h+42)
    w0r = np.zeros((128, 128), np.float32)
    for g in range(4):
        cs = 0 if g % 2 == 0 else 64
        for t in range(9):
            w0r[32 * g + t, cs:cs + 64] = w0[:, 0, t // 3, t % 3]
        w0r[32 * g + 9, cs:cs + 64] = b0
    ws_["w0r"] = w0r

    # L1: kin=1 (Ci=64) blocks replicated into rows 64:128 for b-in-partition
    wb1 = wblocks(cp[2])                       # [9, 128, 128], rows 64: zero
    wb1[:, 64:128, :] = wb1[:, 0:64, :]
    ws_["wl1"] = wb1
    ws_["wl2"] = wblocks(cp[4])                # [18, 128, 128]
    ws_["wl3"] = wblocks(cp[6])                # [36, 128, 128]
    ws_["wl4"] = wblocks(cp[8])                # [72, 128, 128]
    ws_["wl5"] = wblocks(cp[10])               # [144, 128, 128]
    ws_["wl6"] = wblocks(cp[12])               # [64, 128, 128]

    # conv biases as rows (bias-matmul lhsT), concat [1, 128+256+512]
    ws_["bias_rows"] = np.concatenate([cp[3], cp[7], cp[11]])[None, :]
    # BN gamma/beta column-major [128, mt]
    for nm, arr in [("g2", g2), ("bt2", bt2), ("g4", g4), ("bt4", bt4),
                    ("g6", g6), ("bt6", bt6)]:
        ws_[nm] = arr.reshape(-1, 128).T.copy()

    ws_["ones_l0"] = np.ones((1, 32 * 512), np.float32)
    ws_["zeros_l0"] = np.zeros((1, 32 * 512), np.float32)
    ws_["ones_row"] = np.ones((1, 512), np.float32)
    ws_["ones_col"] = np.ones((128, 1), np.float32)

    # decoder
    ws_["emb_tab"] = emb_tab                       # [246, 128]
    ws_["WiT"] = Wi.T.reshape(4, 128, 128).copy()  # k-tiles of Wi.T [512,128]
    ws_["WhT"] = Wh.T.copy()                       # [128, 128]
    ws_["bhT"] = bh[:, None].copy()                # [128, 1]
    ws_["ws_col"] = np.repeat(ws[:, None], 2, axis=1).copy()  # [128, 2]
    ws_["WihcT"] = Wih[:, :SRC].T.reshape(4, 128, 512).copy()
    ws_["WiheT"] = Wih[:, SRC:].T.copy()           # [128, 512]
    ws_["WhhT"] = Whh.T.copy()                     # [128, 512]
    ws_["bih_row"] = bih[None, :].copy()           # [1, 512]
    ws_["bhh_row"] = bhh[None, :].copy()
    WgTp = np.zeros((128, 246), np.float32)
    WgTp[:, 0:245] = Wg.T
    ws_["WgT"] = WgTp
    bgp = np.zeros((1, 246), np.float32)
    bgp[0, 0:245] = bg
    ws_["bg_row"] = bgp

    text = np.asarray(text).astype(np.int32)
    in_maps = []
    for c in range(N_CORES):
        m = dict(ws_)
        m["image"] = np.asarray(image[BPC * c:BPC * c + BPC], np.float32)
        tx = text[BPC * c:BPC * c + BPC]
        idx = np.empty((2 * T_STEPS, 1), np.int32)
        for t in range(T_STEPS):
            idx[2 * t + 0, 0] = tx[0, t]
            idx[2 * t + 1, 0] = tx[1, t]
        m["idx"] = idx
        in_maps.append(m)
    return in_maps


# ------------------------------------------------------------------- builder
class Pools:
    """Explicitly-scoped tile pools so SBUF frees as phases end."""

    def __init__(self, tc):
        self.tc = tc
        self._cms = {}
        self.p = {}

    def open(self, name, bufs=1, space="SBUF", side=None):
        cm = self.tc.tile_pool(name=name, bufs=bufs, space=space, side=side)
        self.p[name] = cm.__enter__()
        self._cms[name] = cm
        return self.p[name]

    def close(self, name):
        self._cms.pop(name).__exit__(None, None, None)
        del self.p[name]

    def close_all(self):
        for name in reversed(list(self._cms)):
            self.close(name)


def build_kernel(debug=False):
    nc = bacc.Bacc("TRN2", target_bir_lowering=False, debug=False,
                   num_devices=N_CORES)

    def din(name, shape, dt=F32):
        return nc.dram_tensor(name, shape, dt, kind="ExternalInput").ap()

    img_d = din("image", [BPC, 1, 128, 512])
    w0r_d = din("w0r", [128, 128])
    wl_d = {1: din("wl1", [9, 128, 128]), 2: din("wl2", [18, 128, 128]),
            3: din("wl3", [36, 128, 128]), 4: din("wl4", [72, 128, 128]),
            5: din("wl5", [144, 128, 128]), 6: din("wl6", [64, 128, 128])}
    bias_rows_d = din("bias_rows", [1, 128 + 256 + 512])
    bn_d = {nm: din(nm, [128, mt]) for nm, mt in
            [("g2", 2), ("bt2", 2), ("g4", 4), ("bt4", 4),
             ("g6", 4), ("bt6", 4)]}
    ones_l0_d = din("ones_l0", [1, 32 * 512])
    zeros_l0_d = din("zeros_l0", [1, 32 * 512])
    ones_row_d = din("ones_row", [1, 512])
    ones_col_d = din("ones_col", [128, 1])
    emb_d = din("emb_tab", [VOCAB, 128])
    idx_d = din("idx", [2 * T_STEPS, 1], I32)
    WiT_d = din("WiT", [4, 128, 128])
    WhT_d = din("WhT", [128, 128])
    bhT_d = din("bhT", [128, 1])
    wscol_d = din("ws_col", [128, 2])
    WihcT_d = din("WihcT", [4, 128, 512])
    WiheT_d = din("WiheT", [128, 512])
    WhhT_d = din("WhhT", [128, 512])
    bih_d = din("bih_row", [1, 512])
    bhh_d = din("bhh_row", [1, 512])
    WgT_d = din("WgT", [128, NCLS + 1])
    bg_d = din("bg_row", [1, NCLS + 1])

    out_d = nc.dram_tensor("out", [2 * T_STEPS, NCLS], F32,
                           kind="ExternalOutput").ap()
    dbg = {}
    if debug:
        for nm, shape in [("dX2", [128, 66, 258]), ("dX3", [128, 2, 34, 130]),
                          ("dX4", [2, 128, 2, 34, 130]),
                          ("dX5", [2, 128, 2, 18, 132]),
                          ("dX6", [4, 128, 2, 18, 132]),
                          ("dX7", [4, 128, 2, 8, 131]),
                          ("dsrcT", [4, 128, 1806]),
                          ("dsf", [128, 1806]),
                          ("dWS", [8, 128, 1024]),
                          ("de0", [128, 1806]),
                          ("dal0", [128, 16]),
                          ("dhs", [128, 98])]:
            dbg[nm] = nc.dram_tensor(nm, shape, F32, kind="ExternalOutput").ap()

    with tile.TileContext(nc) as tc:
        P = Pools(tc)
        _build_body(nc, tc, P, locals())
    nc.compile()
    return nc



def _dzero(nc, g, dst):
    """DMA zeros into an SBUF AP (any partition base / strides), replicating
    the zeros_l0 input across partitions. Needed because Memset can't encode
    float32r."""
    dims = dst.shape
    free = 1
    for d in dims[1:]:
        free *= int(d)
    assert free <= 16384, free
    ap_list = [[0, int(dims[0])]]
    strides = []
    stride = 1
    for d in reversed(dims[1:]):
        strides.append((stride, int(d)))
        stride *= int(d)
    for s, d in reversed(strides):
        ap_list.append([s, d])
    zt = g["zeros_l0_d"]
    src_ap = bass.AP(tensor=zt.tensor, offset=0, ap=ap_list)
    if dst.dtype == R:
        src_ap = src_ap.bitcast(R)
    nc.sync.dma_start(dst, src_ap)


def _build_body(nc, tc, P, g):
    debug = bool(g["dbg"])
    dbg = g["dbg"]
    misc = P.open("misc", bufs=1)

    # ---------- small resident tensors
    ones_row = misc.tile([1, 512], R, name="ones_row")
    nc.sync.dma_start(ones_row[:], g["ones_row_d"].bitcast(R))
    ones_col = misc.tile([128, 1], R, name="ones_col")
    nc.sync.dma_start(ones_col[:], g["ones_col_d"].bitcast(R))
    bias_rows = misc.tile([1, 896], R, name="bias_rows")
    nc.sync.dma_start(bias_rows[:], g["bias_rows_d"].bitcast(R))
    b1_row = bias_rows[:, 0:128]
    b3_row = lambda m: bias_rows[:, 128 + 128 * m:128 + 128 * (m + 1)]
    b5_row = lambda m: bias_rows[:, 384 + 128 * m:384 + 128 * (m + 1)]

    # =====================================================================
    # L0: conv1 (1->64, 3x3 p1) + relu + maxpool 2x2 -> X2
    # X2 partitions = 64*b + c ; logical [66, 258] (rows 1..64, cols 1..256)
    # =====================================================================
    px2 = P.open("px2")
    X2 = px2.tile([128, 66, 258], R, name="X2")
    for bd in (X2[:, 0:1, :], X2[:, 65:66, :], X2[:, :, 0:1],
               X2[:, :, 257:258]):
        _dzero(nc, g, bd)

    pl0 = P.open("pl0")
    buf = pl0.tile([128, 32, 512], R, name="l0buf")
    for gi in range(4):
        for t in range(9):
            part = 32 * gi + t
            if t % 3 == 0:
                _dzero(nc, g, buf[part:part + 1, :, 0:1])
            if t % 3 == 2:
                _dzero(nc, g, buf[part:part + 1, :, 511:512])
    w0r = misc.tile([128, 128], R, name="w0r")
    nc.sync.dma_start(w0r[:], g["w0r_d"].bitcast(R))
    for gi in range(4):  # ones rows for the bias tap
        nc.sync.dma_start(buf[32 * gi + 9:32 * gi + 10, :, :],
                          g["ones_l0_d"].bitcast(R).rearrange(
                              "o (r c) -> o r c", r=32))

    ps_l0 = P.open("ps_l0", bufs=4, space="PSUM")
    sc_l0 = P.open("sc_l0", bufs=3)
    img = g["img_d"]
    for p in range(2):
        # stage the 4 (b, h) strips: partitions 32*(2b+h)+tap
        for b in range(BPC):
            for h in range(2):
                gi = 2 * h + b
                Y0 = 64 * p + 32 * h
                for t in range(9):
                    dy, dx = t // 3, t % 3
                    r_lo = max(0, 1 - dy - Y0)
                    r_hi = min(32, 129 - dy - Y0)
                    c_lo = max(0, 1 - dx)
                    c_hi = min(512, 513 - dx)
                    part = 32 * gi + t
                    zl0 = g["zeros_l0_d"].bitcast(R).rearrange(
                        "o (r c) -> o r c", r=32)
                    if r_lo > 0:
                        nc.sync.dma_start(buf[part:part + 1, 0:r_lo, :],
                                          zl0[:, 0:r_lo, :])
                    if r_hi < 32:
                        nc.sync.dma_start(buf[part:part + 1, r_hi:32, :],
                                          zl0[:, r_hi:32, :])
                    nc.sync.dma_start(
                        buf[part:part + 1, r_lo:r_hi, c_lo:c_hi],
                        img[b:b + 1, 0, Y0 + dy - 1 + r_lo:Y0 + dy - 1 + r_hi,
                            c_lo + dx - 1:c_hi + dx - 1].bitcast(R))
        for h in range(2):
            for pr in range(16):
                ps = ps_l0.tile([128, 1024], F32, name=_nm("l0ps"), tag="l0ps")
                for rr in range(2):
                    nc.tensor.matmul(
                        ps[:, 512 * rr:512 * rr + 512],
                        lhsT=w0r[64 * h:64 * h + 42, :],
                        rhs=buf[64 * h:64 * h + 42, 2 * pr + rr, :],
                        start=True, stop=True)
                rf = sc_l0.tile([128, 1024], F32, name=_nm("l0rf"),
                                tag="l0rf")
                nc.scalar.activation(rf[:], ps[:], AF.Relu)
                rm = sc_l0.tile([128, 512], R, name=_nm("l0rm"), tag="l0rm")
                nc.vector.tensor_tensor(out=rm[:], in0=rf[:, 0:512],
                                        in1=rf[:, 512:1024], op=OP.max)
                yy = 32 * p + 16 * h + pr
                nc.vector.scalar_tensor_tensor(
                    out=X2[:, 1 + yy, 1:257],
                    in0=rm[:, 0:512:2], scalar=0.0, in1=rm[:, 1:512:2],
                    op0=OP.bypass, op1=OP.max)
    P.close("ps_l0"); P.close("sc_l0"); P.close("pl0")
    if debug:
        nc.sync.dma_start(dbg["dX2"], X2[:].bitcast(F32))

    # =====================================================================
    # L1: conv2 (64->128) + relu + maxpool 2x2 -> X3 [128, 2, 34, 130]
    # b-in-partition trick: b0 uses K rows 0:64, b1 rows 64:128 (concurrent)
    # =====================================================================
    px3 = P.open("px3", side="right")
    X3 = px3.tile([128, 2, 34, 130], R, name="X3")
    for bd in (X3[:, :, 0:1, :], X3[:, :, 33:34, :], X3[:, :, :, 0:1],
               X3[:, :, :, 129:130]):
        _dzero(nc, g, bd)
    w1 = misc.tile([128, 9, 128], R, name="w1")
    nc.sync.dma_start(w1[:], g["wl_d"][1].bitcast(R).rearrange("n p c -> p n c"))

    ps_l1 = P.open("ps_l1", bufs=8, space="PSUM")
    sc_l1 = P.open("sc_l1", bufs=3)
    for pr in range(32):
        for b in range(BPC):
            ps = ps_l1.tile([128, 512], F32, name=_nm("l1ps"), tag="l1ps")
            for rr in range(2):
                yy = 2 * pr + rr
                for t in range(9):
                    dy, dx = t // 3, t % 3
                    nc.tensor.matmul(
                        ps[:, 256 * rr:256 * rr + 256],
                        lhsT=w1[64 * b:64 * b + 64, t, :],
                        rhs=X2[64 * b:64 * b + 64, yy + dy, dx:dx + 256],
                        start=(t == 0), stop=False)
                nc.tensor.matmul(
                    ps[:, 256 * rr:256 * rr + 256],
                    lhsT=b1_row, rhs=ones_row[:, 0:256],
                    start=False, stop=True)
            rf = sc_l1.tile([128, 512], F32, name=_nm("l1rf"), tag="l1rf")
            nc.scalar.activation(rf[:], ps[:], AF.Relu)
            rm = sc_l1.tile([128, 256], R, name=_nm("l1rm"), tag="l1rm")
            nc.vector.tensor_tensor(out=rm[:], in0=rf[:, 0:256],
                                    in1=rf[:, 256:512], op=OP.max)
            nc.vector.scalar_tensor_tensor(
                out=X3[:, b, 1 + pr, 1:129],
                in0=rm[:, 0:256:2], scalar=0.0, in1=rm[:, 1:256:2],
                op0=OP.bypass, op1=OP.max)
    P.close("ps_l1"); P.close("sc_l1"); P.close("px2")
    if debug:
        nc.sync.dma_start(dbg["dX3"], X3[:].bitcast(F32))

    # =====================================================================
    # L2: conv3 (128->256) -> raw ; BN2(allreduce) + relu -> X4
    # =====================================================================
    w2 = misc.tile([128, 18, 128], R, name="w2")
    nc.sync.dma_start(w2[:], g["wl_d"][2].bitcast(R).rearrange("n p c -> p n c"))
    praw2 = P.open("praw2")
    raw2 = [praw2.tile([128, 2 * 32 * 128], F32, name=f"raw2_{m}")
            for m in range(2)]
    ps_l2 = P.open("ps_l2", bufs=8, space="PSUM")
    for b in range(BPC):
        for pr in range(16):
            for m in range(2):
                ps = ps_l2.tile([128, 256], F32, name=_nm("l2ps"), tag="l2ps")
                for t in range(9):
                    dy, dx = t // 3, t % 3
                    nc.tensor.matmul(
                        ps[:],
                        lhsT=w2[:, 9 * m + t, :],
                        rhs=X3[:, b, 2 * pr + dy:2 * pr + dy + 2, dx:dx + 128],
                        start=(t == 0), stop=(t == 8))
                nc.scalar.activation(
                    raw2[m][:, 4096 * b + 256 * pr:4096 * b + 256 * pr + 256],
                    ps[:], AF.Copy)
    P.close("ps_l2"); P.close("px3")

    px4 = P.open("px4", side="right")
    X4 = [px4.tile([128, 2, 34, 130], R, name=f"X4_{m}") for m in range(2)]
    for x in X4:
        for bd in (x[:, :, 0:1, :], x[:, :, 33:34, :], x[:, :, :, 0:1],
                   x[:, :, :, 129:130]):
            _dzero(nc, g, bd)
    _bn_relu(nc, tc, P, g, layer=2, raws=raw2, S=8192, chunk=512,
             gamma="g2", beta="bt2",
             outs=[x[:, :, 1:33, 1:129] for x in X4], apply_shape=(2, 32, 128))
    P.close("praw2")
    if debug:
        for m in range(2):
            nc.sync.dma_start(dbg["dX4"][m], X4[m][:].bitcast(F32))

    # =====================================================================
    # L3: conv4 (256->256) + relu + pool (2,2)s(2,1)p(0,1) -> X5
    # =====================================================================
    px5 = P.open("px5")
    X5 = [px5.tile([128, 2, 18, 132], R, name=f"X5_{m}") for m in range(2)]
    for x in X5:
        for bd in (x[:, :, 0:1, :], x[:, :, 17:18, :], x[:, :, :, 0:1],
                   x[:, :, :, 130:132]):
            _dzero(nc, g, bd)
    _conv_pool_s1(nc, P, g, lidx=3, Xin=X4, Xout=X5, kin=2, mt=2,
                  H=32, W=128, bias_row=b3_row, wd=g["wl_d"][3], rounds=4,
                  ones_row=ones_row)
    P.close("px4")
    if debug:
        for m in range(2):
            nc.sync.dma_start(dbg["dX5"][m], X5[m][:].bitcast(F32))

    # =====================================================================
    # L4: conv5 (256->512) -> raw ; BN4 + relu -> X6
    # =====================================================================
    praw4 = P.open("praw4")
    raw4 = [praw4.tile([128, 2 * 16 * 129], F32, name=f"raw4_{m}")
            for m in range(4)]
    _conv_raw(nc, P, g, lidx=4, Xin=X5, raws=raw4, kin=2, mt=4,
              H=16, W=129, wd=g["wl_d"][4], rounds=2)
    px6 = P.open("px6", side="right")
    X6 = [px6.tile([128, 2, 18, 132], R, name=f"X6_{m}") for m in range(4)]
    for x in X6:
        for bd in (x[:, :, 0:1, :], x[:, :, 17:18, :], x[:, :, :, 0:1],
                   x[:, :, :, 130:132]):
            _dzero(nc, g, bd)
    _bn_relu(nc, tc, P, g, layer=4, raws=raw4, S=4128, chunk=129,
             gamma="g4", beta="bt4",
             outs=[x[:, :, 1:17, 1:130] for x in X6], apply_shape=(2, 16, 129))
    P.close("praw4"); P.close("px5")
    if debug:
        for m in range(4):
            nc.sync.dma_start(dbg["dX6"][m], X6[m][:].bitcast(F32))

    # =====================================================================
    # L5: conv6 (512->512) + relu + pool (2,2)s(2,1)p(0,1) -> X7
    # X7 has no padding (L6 is 2x2 pad-0): [128, 2, 8, 130]
    # =====================================================================
    px7 = P.open("px7")
    X7 = [px7.tile([128, 2, 8, 131], R, name=f"X7_{m}") for m in range(4)]
    for x in X7:
        _dzero(nc, g, x[:, :, :, 130:131])
    _conv_pool_s1(nc, P, g, lidx=5, Xin=X6, Xout=X7, kin=4, mt=4,
                  H=16, W=129, bias_row=b5_row, wd=g["wl_d"][5], rounds=2,
                  out_off=0, ones_row=ones_row)
    P.close("px6")
    if debug:
        for m in range(4):
            nc.sync.dma_start(dbg["dX7"][m], X7[m][:].bitcast(F32))

    # =====================================================================
    # L6: conv7 (512->512, 2x2 p0) -> raw ; BN6 + relu -> srcT
    # srcT[m] = [128, (b,l)] with l = y*129+x, directly the decoder layout
    # =====================================================================
    praw6 = P.open("praw6", side="right")
    raw6 = [praw6.tile([128, 2 * L], F32, name=f"raw6_{m}") for m in range(4)]
    wp = P.open("w6s", bufs=4)
    ps6 = P.open("ps_l6", bufs=8, space="PSUM")
    ygroups = [(0, 2), (2, 2), (4, 2), (6, 1)]
    for m in range(4):
        pss = {}
        for b in range(BPC):
            for gy, (y0, nr) in enumerate(ygroups):
                pss[b, gy] = ps6.tile([128, 260], F32, name=_nm("l6ps"),
                                      tag="l6ps")
        for k in range(4):
            for t in range(4):
                dy, dx = t // 2, t % 2
                w_sb = wp.tile([128, 128], R, name=_nm("w6"), tag="w6w")
                nc.sync.dma_start(
                    w_sb[:], g["wl_d"][6][16 * m + 4 * k + t].bitcast(R))
                for b in range(BPC):
                    for gy, (y0, nr) in enumerate(ygroups):
                        nc.tensor.matmul(
                            pss[b, gy][:, 0:130 * nr],
                            lhsT=w_sb[:],
                            rhs=X7[k][:, b, y0 + dy:y0 + dy + nr, dx:dx + 130],
                            start=(k == 0 and t == 0),
                            stop=(k == 3 and t == 3))
        for b in range(BPC):
            for gy, (y0, nr) in enumerate(ygroups):
                src_v = pss[b, gy][:, 0:130 * nr].rearrange(
                    "p (r w) -> p r w", r=nr)[:, :, 0:129]
                dst = raw6[m][:, b * L + 129 * y0: b * L + 129 * (y0 + nr)]
                nc.scalar.activation(
                    dst.rearrange("p (r w) -> p r w", r=nr), src_v, AF.Copy)
    P.close("ps_l6"); P.close("w6s"); P.close("px7")

    psrc = P.open("psrc")
    srcT = [psrc.tile([128, 2 * L], R, name=f"srcT_{m}") for m in range(4)]
    _bn_relu(nc, tc, P, g, layer=6, raws=raw6, S=2 * L, chunk=301,
             gamma="g6", beta="bt6", outs=[s[:] for s in srcT])
    P.close("praw6")
    if debug:
        for m in range(4):
            nc.sync.dma_start(dbg["dsrcT"][m], srcT[m][:].bitcast(F32))

    # =====================================================================
    # Decoder
    # =====================================================================
    _decoder(nc, tc, P, g, srcT, misc, ones_row, ones_col, dbg, debug)
    P.close_all()


def _conv_raw(nc, P, g, lidx, Xin, raws, kin, mt, H, W, wd, rounds):
    """BN-style conv: 3x3 pad-1, 2-row-packed N, ACT Copy evac to raw.

    f32r moving-AP dims must have even counts, so odd widths W compute an
    even window Wm = W+1 (one garbage column per row, skipped at evac)."""
    Wm = W + (W % 2)
    wp = P.open(f"w{lidx}s", bufs=4)
    psp = P.open(f"ps_l{lidx}", bufs=8, space="PSUM")
    pairs = [(b, pr) for b in range(BPC) for pr in range(H // 2)]
    per = len(pairs) // rounds
    for m in range(mt):
        for rnd in range(rounds):
            chunk_prs = pairs[rnd * per:(rnd + 1) * per]
            pss = {bp: psp.tile([128, 2 * Wm], F32, name=_nm(f"l{lidx}ps"),
                                tag=f"l{lidx}ps") for bp in chunk_prs}
            for k in range(kin):
                for t in range(9):
                    dy, dx = t // 3, t % 3
                    w_sb = wp.tile([128, 128], R, name=_nm(f"w{lidx}"),
                                   tag=f"w{lidx}w")
                    nc.sync.dma_start(
                        w_sb[:], wd[kin * 9 * m + 9 * k + t].bitcast(R))
                    for (b, pr) in chunk_prs:
                        nc.tensor.matmul(
                            pss[b, pr][:],
                            lhsT=w_sb[:],
                            rhs=Xin[k][:, b, 2 * pr + dy:2 * pr + dy + 2,
                                       dx:dx + Wm],
                            start=(k == 0 and t == 0),
                            stop=(k == kin - 1 and t == 8))
            for (b, pr) in chunk_prs:
                src_v = pss[b, pr][:].rearrange(
                    "p (r w) -> p r w", r=2)[:, :, 0:W]
                dst = raws[m][:, H * W * b + 2 * W * pr:
                              H * W * b + 2 * W * pr + 2 * W]
                nc.scalar.activation(
                    dst.rearrange("p (r w) -> p r w", r=2), src_v, AF.Copy)
    P.close(f"ps_l{lidx}"); P.close(f"w{lidx}s")


def _conv_pool_s1(nc, P, g, lidx, Xin, Xout, kin, mt, H, W, bias_row, wd,
                  rounds, ones_row, out_off=1):
    """Pool-style conv: 3x3 pad-1 + relu + maxpool (2,2)s(2,1)p(0,1).

    Pair rows packed in psum halves; bias added via an extra ones-matmul;
    DVE stt does relu+pair-max; gpsimd does the stride-1 W-max; edge cols
    copied. Output interior starts at (row out_off, col out_off).
    """
    Wo = W + 1  # pooled width
    Wm = W + (W % 2)  # even f32r window; extra col is garbage, skipped below
    wp = P.open(f"w{lidx}s", bufs=4)
    psp = P.open(f"ps_l{lidx}", bufs=8, space="PSUM")
    scp = P.open(f"sc_l{lidx}", bufs=3)
    pairs = [(b, pr) for b in range(BPC) for pr in range(H // 2)]
    per = len(pairs) // rounds
    for m in range(mt):
        for rnd in range(rounds):
            chunk_prs = pairs[rnd * per:(rnd + 1) * per]
            pss = {bp: psp.tile([128, 2 * Wm], F32, name=_nm(f"l{lidx}ps"),
                                tag=f"l{lidx}ps") for bp in chunk_prs}
            for k in range(kin):
                for t in range(9):
                    dy, dx = t // 3, t % 3
                    w_sb = wp.tile([128, 128], R, name=_nm(f"w{lidx}"),
                                   tag=f"w{lidx}w")
                    nc.sync.dma_start(
                        w_sb[:], wd[kin * 9 * m + 9 * k + t].bitcast(R))
                    for (b, pr) in chunk_prs:
                        nc.tensor.matmul(
                            pss[b, pr][:],
                            lhsT=w_sb[:],
                            rhs=Xin[k][:, b, 2 * pr + dy:2 * pr + dy + 2,
                                       dx:dx + Wm],
                            start=(k == 0 and t == 0), stop=False)
            for (b, pr) in chunk_prs:
                # bias into both row-halves, then close the group
                nc.tensor.matmul(pss[b, pr][:, 0:Wm], lhsT=bias_row(m),
                                 rhs=ones_row[:, 0:Wm],
                                 start=False, stop=False)
                nc.tensor.matmul(pss[b, pr][:, Wm:2 * Wm], lhsT=bias_row(m),
                                 rhs=ones_row[:, 0:Wm],
                                 start=False, stop=True)
                rf = scp.tile([128, 2 * Wm], F32, name=_nm(f"l{lidx}rf"),
                              tag=f"l{lidx}rf")
                nc.scalar.activation(rf[:], pss[b, pr][:], AF.Relu)
                rm = scp.tile([128, W], R, name=_nm(f"l{lidx}rm"),
                              tag=f"l{lidx}rm")
                nc.vector.tensor_tensor(out=rm[:], in0=rf[:, 0:W],
                                        in1=rf[:, Wm:Wm + W], op=OP.max)
                # W-pool stride 1 pad 1: out[x]=max(rm[x-1], rm[x])
                yy = out_off + pr
                nc.vector.scalar_tensor_tensor(
                    out=Xout[m][:, b, yy, out_off + 1:out_off + W],
                    in0=rm[:, 0:W - 1], scalar=0.0, in1=rm[:, 1:W],
                    op0=OP.bypass, op1=OP.max)
                # edges: out[0]=rm[0], out[W]=rm[W-1]
                nc.vector.tensor_copy(
                    Xout[m][:, b, yy, out_off:out_off + W + 1:W],
                    rm[:, 0:W:W - 1])
    P.close(f"ps_l{lidx}"); P.close(f"sc_l{lidx}"); P.close(f"w{lidx}s")


def _bn_relu(nc, tc, P, g, layer, raws, S, chunk, gamma, beta, outs,
             apply_shape=None):
    """Train-mode BN with cross-core stats + fused relu apply."""
    mt = len(raws)
    pbn = P.open(f"bn{layer}", bufs=1)
    dramp = P.open(f"bnd{layer}", bufs=1, space="DRAM")
    nch = S // chunk
    payload = pbn.tile([128, 2 * mt], F32, name=f"pay{layer}")
    for m in range(mt):
        stats = pbn.tile([128, nch * 6], F32, name=_nm(f"st{layer}"),
                         tag=f"st{layer}")
        for i in range(nch):
            nc.vector.bn_stats(
                out=stats[:, 6 * i:6 * i + 6],
                in_=raws[m][:, chunk * i:chunk * (i + 1)])
        mv = pbn.tile([128, 2], F32, name=_nm(f"mv{layer}"), tag=f"mv{layer}")
        nc.vector.bn_aggr(out=mv[:], in_=stats[:])
        # payload: [sum, sumsq] = [mean*S, (var+mean^2)*S]
        msq = pbn.tile([128, 1], F32, name=_nm(f"msq{layer}"),
                       tag=f"msq{layer}")
        nc.vector.tensor_tensor(out=msq[:], in0=mv[:, 0:1], in1=mv[:, 0:1],
                                op=OP.mult)
        nc.vector.tensor_scalar_mul(payload[:, 2 * m:2 * m + 1],
                                    mv[:, 0:1], float(S))
        ex2 = pbn.tile([128, 1], F32, name=_nm(f"ex2{layer}"),
                       tag=f"ex2{layer}")
        nc.vector.tensor_tensor(out=ex2[:], in0=mv[:, 1:2], in1=msq[:],
                                op=OP.add)
        nc.vector.tensor_scalar_mul(payload[:, 2 * m + 1:2 * m + 2],
                                    ex2[:], float(S))
    cc_in = dramp.tile([128, 2 * mt], F32, name=f"ccin{layer}")
    cc_out = dramp.tile([128, 2 * mt], F32, name=f"ccout{layer}",
                        addr_space="Shared")
    nc.sync.dma_start(cc_in[:], payload[:])
    nc.gpsimd.collective_compute(
        "AllReduce", OP.add, replica_groups=[list(range(N_CORES))],
        ins=[cc_in[:].opt()], outs=[cc_out[:].opt()])
    gsum = pbn.tile([128, 2 * mt], F32, name=f"gsum{layer}")
    nc.sync.dma_start(gsum[:], cc_out[:])

    gam = pbn.tile([128, mt], F32, name=f"gam{layer}")
    nc.sync.dma_start(gam[:], g["bn_d"][gamma])
    bet = pbn.tile([128, mt], F32, name=f"bet{layer}")
    nc.sync.dma_start(bet[:], g["bn_d"][beta])

    Ntot = float(S * N_CORES)
    for m in range(mt):
        mean = pbn.tile([128, 1], F32, name=_nm(f"mean{layer}"),
                        tag=f"mean{layer}")
        nc.vector.tensor_scalar_mul(mean[:], gsum[:, 2 * m:2 * m + 1],
                                    1.0 / Ntot)
        ex2 = pbn.tile([128, 1], F32, name=_nm(f"gex2{layer}"),
                       tag=f"gex2{layer}")
        nc.vector.tensor_scalar_mul(ex2[:], gsum[:, 2 * m + 1:2 * m + 2],
                                    1.0 / Ntot)
        msq = pbn.tile([128, 1], F32, name=_nm(f"gmsq{layer}"),
                       tag=f"gmsq{layer}")
        nc.vector.tensor_tensor(out=msq[:], in0=mean[:], in1=mean[:],
                                op=OP.mult)
        varps = pbn.tile([128, 1], F32, name=_nm(f"var{layer}"),
                         tag=f"var{layer}")
        nc.vector.tensor_tensor(out=varps[:], in0=ex2[:], in1=msq[:],
                                op=OP.subtract)
        nc.vector.tensor_scalar_add(varps[:], varps[:], 1e-5)
        # s = sqrt(varps); one Newton step; rstd = 1/s
        s0 = pbn.tile([128, 1], F32, name=_nm(f"s0{layer}"),
                      tag=f"s0{layer}")
        nc.scalar.activation(s0[:], varps[:], AF.Sqrt)
        r0 = pbn.tile([128, 1], F32, name=_nm(f"r0{layer}"),
                      tag=f"r0{layer}")
        nc.vector.reciprocal(r0[:], s0[:])
        xr = pbn.tile([128, 1], F32, name=_nm(f"xr{layer}"),
                      tag=f"xr{layer}")
        nc.vector.tensor_tensor(out=xr[:], in0=varps[:], in1=r0[:],
                                op=OP.mult)
        s1 = pbn.tile([128, 1], F32, name=_nm(f"s1{layer}"),
                      tag=f"s1{layer}")
        nc.vector.tensor_tensor(out=s1[:], in0=s0[:], in1=xr[:], op=OP.add)
        nc.vector.tensor_scalar_mul(s1[:], s1[:], 0.5)
        rstd = pbn.tile([128, 1], F32, name=_nm(f"rstd{layer}"),
                        tag=f"rstd{layer}")
        nc.vector.reciprocal(rstd[:], s1[:])
        a = pbn.tile([128, 1], F32, name=_nm(f"a{layer}"), tag=f"a{layer}")
        nc.vector.tensor_tensor(out=a[:], in0=gam[:, m:m + 1], in1=rstd[:],
                                op=OP.mult)
        ma = pbn.tile([128, 1], F32, name=_nm(f"ma{layer}"), tag=f"ma{layer}")
        nc.vector.tensor_tensor(out=ma[:], in0=mean[:], in1=a[:], op=OP.mult)
        bc = pbn.tile([128, 1], F32, name=_nm(f"bc{layer}"), tag=f"bc{layer}")
        nc.vector.tensor_tensor(out=bc[:], in0=bet[:, m:m + 1], in1=ma[:],
                                op=OP.subtract)
        in_ap = raws[m][:]
        if apply_shape is not None:
            bb, yy_, xx = apply_shape
            in_ap = in_ap.rearrange("p (b y x) -> p b y x", b=bb, y=yy_)
        nc.scalar.activation(outs[m], in_ap, AF.Relu,
                             bias=bc[:], scale=a[:])
    P.close(f"bnd{layer}"); P.close(f"bn{layer}")


def _decoder(nc, tc, P, g, srcT, misc, ones_row, ones_col, dbg, debug):
    pd = P.open("dec", bufs=1)
    psd = P.open("dec_ps", bufs=1, space="PSUM")

    ident = misc.tile([128, 128], F32, name="ident")
    make_identity(nc, ident[:])

    # ---- embedding gather + transpose -> embT [128, 98] (cols t*2+b)
    idx_sb = pd.tile([2 * T_STEPS, 1], I32, name="idx_sb")
    nc.sync.dma_start(idx_sb[:], g["idx_d"])
    gath = pd.tile([2 * T_STEPS, 128], F32, name="gath")
    nc.gpsimd.indirect_dma_start(
        out=gath[:], out_offset=None, in_=g["emb_d"],
        in_offset=bass.IndirectOffsetOnAxis(ap=idx_sb[:, 0:1], axis=0))
    embT_ps = psd.tile([128, 2 * T_STEPS], F32, name="embT_ps", tag="preps", bufs=2)
    nc.tensor.transpose(embT_ps[:], gath[:], ident[0:2 * T_STEPS, 0:2 * T_STEPS])
    embT = pd.tile([128, 2 * T_STEPS], R, name="embT")
    nc.vector.tensor_copy(embT[:], embT_ps[:])

    # ---- decoder weights
    WiT = pd.tile([128, 4, 128], R, name="WiT")
    nc.sync.dma_start(WiT[:], g["WiT_d"].bitcast(R).rearrange("n p c -> p n c"))
    WhT = pd.tile([128, 128], R, name="WhT")
    nc.sync.dma_start(WhT[:], g["WhT_d"].bitcast(R))
    bhT = pd.tile([128, 1], F32, name="bhT")
    nc.sync.dma_start(bhT[:], g["bhT_d"])
    ws_col = pd.tile([128, 2], R, name="ws_col")
    nc.sync.dma_start(ws_col[:], g["wscol_d"].bitcast(R))
    WihcT = pd.tile([128, 4, 512], R, name="WihcT")
    nc.sync.dma_start(WihcT[:],
                      g["WihcT_d"].bitcast(R).rearrange("n p c -> p n c"))
    WiheT = pd.tile([128, 512], R, name="WiheT")
    nc.sync.dma_start(WiheT[:], g["WiheT_d"].bitcast(R))
    WhhT = pd.tile([128, 512], R, name="WhhT")
    nc.sync.dma_start(WhhT[:], g["WhhT_d"].bitcast(R))
    bih = pd.tile([1, 512], F32, name="bih")
    nc.sync.dma_start(bih[:], g["bih_d"])
    bhh = pd.tile([1, 512], F32, name="bhh")
    nc.sync.dma_start(bhh[:], g["bhh_d"])
    bias_g = pd.tile([1, 512], R, name="bias_g")
    nc.vector.tensor_tensor(out=bias_g[:], in0=bih[:], in1=bhh[:], op=OP.add)
    WgT = pd.tile([128, NCLS + 1], R, name="WgT")
    nc.sync.dma_start(WgT[:], g["WgT_d"].bitcast(R))
    bg_row = pd.tile([1, NCLS + 1], R, name="bg_row")
    nc.sync.dma_start(bg_row[:], g["bg_d"].bitcast(R))

    lchunks = [(i * 128, min(128, L - i * 128)) for i in range(8)]  # 7x128+7

    # ---- src_featT [128, (b,l)] = Wi @ srcT
    sf = pd.tile([128, 2 * L], F32, name="sf")
    for nch in range(4):
        c0 = 452 * nch
        cw = min(452, 2 * L - c0)
        ps = psd.tile([128, 452], F32, name=_nm("sfps"), tag="preps", bufs=2)
        for k in range(4):
            nc.tensor.matmul(ps[:, 0:cw], lhsT=WiT[:, k, :],
                             rhs=srcT[k][:, c0:c0 + cw],
                             start=(k == 0), stop=(k == 3))
        nc.vector.tensor_copy(sf[:, c0:c0 + cw], ps[:, 0:cw])
    if debug:
        nc.sync.dma_start(dbg["dsf"], sf[:].bitcast(F32))

    # ---- WSrc[lc] [128, (b,j)] = Wih_ctx @ src  (attention folded into gates)
    WSrc = [pd.tile([128, 2 * 512], R, name=f"WSrc_{i}") for i in range(8)]
    for b in range(BPC):
        for lc, (l0, lw) in enumerate(lchunks):
            ps = psd.tile([128, 512], F32, name=_nm("wsps"), tag="preps", bufs=2)
            for k in range(4):
                nc.tensor.matmul(
                    ps[0:lw, :], lhsT=srcT[k][:, b * L + l0:b * L + l0 + lw],
                    rhs=WihcT[:, k, :], start=(k == 0), stop=(k == 3))
            nc.scalar.activation(WSrc[lc][0:lw, 512 * b:512 * b + 512],
                                 ps[0:lw, :], AF.Copy)
    if debug:
        for i in range(8):
            nc.sync.dma_start(dbg["dWS"][i], WSrc[i][:].bitcast(F32))

    # ---- recurrent loop
    hsT = pd.tile([128, 2 * T_STEPS], R, name="hsT")
    h0 = pd.tile([128, 2], R, name="h0")
    _dzero(nc, g, h0[:])
    c_b = [pd.tile([1, 128], F32, name=f"c_b{b}") for b in range(BPC)]
    for cb in c_b:
        nc.vector.memset(cb[:], 0.0)
    e_pool = P.open("e_pool", bufs=2)
    st_pool = P.open("st_pool", bufs=2)

    for t in range(T_STEPS):
        hT = h0[:] if t == 0 else hsT[:, t - 1:t + T_STEPS:T_STEPS]
        # proj
        proj_ps = psd.tile([128, 2], F32, name=_nm("proj"), tag="smallps", bufs=1)
        nc.tensor.matmul(proj_ps[:], lhsT=WhT[:], rhs=hT, start=True,
                         stop=True)
        projb = st_pool.tile([128, 2], F32, name=_nm("projb"), tag="projb")
        nc.vector.tensor_scalar(out=projb[:], in0=proj_ps[:], scalar1=bhT[:],
                                scalar2=None, op0=OP.add)
        # e = tanh(sf + projb)
        e = e_pool.tile([128, 2 * L], R, name=_nm("e"), tag="e")
        for b in range(BPC):
            nc.scalar.activation(e[:, b * L:(b + 1) * L],
                                 sf[:, b * L:(b + 1) * L],
                                 AF.Tanh, bias=projb[:, b:b + 1])
        if debug and t == 0:
            nc.sync.dma_start(dbg["de0"], e[:].bitcast(F32))
        # logits (l on partitions): 16 tiny e-stationary matmuls
        log_ps = psd.tile([128, 32], F32, name=_nm("logps"), tag="logps", bufs=2)
        nc.vector.memset(log_ps[:], -1e30)
        for b in range(BPC):
            for lc, (l0, lw) in enumerate(lchunks):
                c0 = 2 * (8 * b + lc)
                nc.tensor.matmul(
                    log_ps[0:lw, c0:c0 + 2],
                    lhsT=e[:, b * L + l0:b * L + l0 + lw],
                    rhs=ws_col[:], start=True, stop=True)
        expv = st_pool.tile([128, 32], R, name=_nm("expv"), tag="expv")
        nc.scalar.activation(expv[:], log_ps[:], AF.Exp)
        # sums over l: ones.T @ expv -> [1, 16]; fold chunks; reciprocal
        sum_ps = psd.tile([1, 32], F32, name=_nm("sumps"), tag="smallps", bufs=1)
        nc.tensor.matmul(sum_ps[:], lhsT=ones_col[:], rhs=expv[:],
                         start=True, stop=True)
        sums2 = st_pool.tile([1, 2], F32, name=_nm("sums2"), tag="sums2")
        nc.vector.tensor_reduce(
            out=sums2[:],
            in_=sum_ps[:].rearrange("o (b c d) -> o b c d", b=2, c=8)[:, :, :, 0],
            axis=mybir.AxisListType.X, op=OP.add)
        recip = st_pool.tile([1, 2], R, name=_nm("recip"), tag="recip")
        with nc.allow_low_precision(reason="f32r is bit-identical to f32"):
            nc.vector.reciprocal(recip[:], sums2[:])
        # broadcast recip across partitions, scale exp -> alpha
        rt_ps = psd.tile([128, 2], F32, name=_nm("rtps"), tag="smallps", bufs=1)
        nc.tensor.matmul(rt_ps[:], lhsT=ones_row[:, 0:128], rhs=recip[:],
                         start=True, stop=True)
        recipT = st_pool.tile([128, 2], F32, name=_nm("recipT"), tag="recipT")
        nc.vector.tensor_copy(recipT[:], rt_ps[:])
        alpha = st_pool.tile([128, 16], R, name=_nm("alpha"), tag="alpha")
        for b in range(BPC):
            nc.vector.tensor_scalar(out=alpha[:, 8 * b:8 * b + 8],
                                    in0=expv[:, 16 * b:16 * b + 16:2],
                                    scalar1=recipT[:, b:b + 1], scalar2=None,
                                    op0=OP.mult)
        if debug and t == 0:
            nc.sync.dma_start(dbg["dal0"], alpha[:].bitcast(F32))
        # gates_b: everything accumulates into one [1, 512] psum per b:
        # 8 ctx MMs + emb MM + h MM + bias MM (partition-0 base throughout)
        for b in range(BPC):
            gps = psd.tile([1, 512], F32, name=_nm(f"gctx{b}"),
                           tag=f"gctx{b}", bufs=1)
            for lc, (l0, lw) in enumerate(lchunks):
                nc.tensor.matmul(
                    gps[:],
                    lhsT=alpha[0:lw, 8 * b + lc:8 * b + lc + 1],
                    rhs=WSrc[lc][0:lw, 512 * b:512 * b + 512],
                    start=(lc == 0), stop=False)
            nc.tensor.matmul(gps[:], lhsT=embT[:, 2 * t + b:2 * t + b + 1],
                             rhs=WiheT[:], start=False, stop=False)
            nc.tensor.matmul(gps[:], lhsT=hT[:, b:b + 1], rhs=WhhT[:],
                             start=False, stop=False)
            nc.tensor.matmul(gps[:], lhsT=ones_row[:, 0:1], rhs=bias_g[:],
                             start=False, stop=True)
            # LSTM pointwise per b (i|f|g|o), all at partition base 0
            sig_if = st_pool.tile([1, 256], F32, name=_nm("sigif"),
                                  tag=f"sigif{b}")
            nc.scalar.activation(sig_if[:], gps[0:1, 0:256], AF.Sigmoid)
            tan_g = st_pool.tile([1, 128], F32, name=_nm("tang"),
                                 tag=f"tang{b}")
            nc.scalar.activation(tan_g[:], gps[0:1, 256:384], AF.Tanh)
            sig_o = st_pool.tile([1, 128], F32, name=_nm("sigo"),
                                 tag=f"sigo{b}")
            nc.scalar.activation(sig_o[:], gps[0:1, 384:512], AF.Sigmoid)
            ig = st_pool.tile([1, 128], F32, name=_nm("ig"), tag=f"ig{b}")
            nc.vector.tensor_tensor(out=ig[:], in0=sig_if[:, 0:128],
                                    in1=tan_g[:], op=OP.mult)
            fc = st_pool.tile([1, 128], F32, name=_nm("fc"), tag=f"fc{b}")
            nc.vector.tensor_tensor(out=fc[:], in0=sig_if[:, 128:256],
                                    in1=c_b[b][:], op=OP.mult)
            nc.vector.tensor_tensor(out=c_b[b][:], in0=ig[:], in1=fc[:],
                                    op=OP.add)
            tan_c = st_pool.tile([1, 128], F32, name=_nm("tanc"),
                                 tag=f"tanc{b}")
            nc.scalar.activation(tan_c[:], c_b[b][:], AF.Tanh)
            h_b = st_pool.tile([1, 128], F32, name=_nm("hb"), tag=f"hb{b}")
            nc.vector.tensor_tensor(out=h_b[:], in0=sig_o[:], in1=tan_c[:],
                                    op=OP.mult)
            ht_ps = psd.tile([128, 2], F32, name=_nm("htps"), tag="smallps",
                             bufs=1)
            nc.tensor.transpose(ht_ps[:], h_b[:], ident[0:1, 0:2])
            nc.vector.tensor_copy(hsT[:, t + T_STEPS * b:t + T_STEPS * b + 1],
                                  ht_ps[:, 0:1])

    if debug:
        nc.sync.dma_start(dbg["dhs"], hsT[:].bitcast(F32))

    # ---- generator: out[(b,t), k] = hsT.T @ WgT + bg
    out_ps = psd.tile([2 * T_STEPS, NCLS + 1], F32, name="out_ps",
                      tag="preps", bufs=2)
    nc.tensor.matmul(out_ps[:], lhsT=ones_row[:, 0:2 * T_STEPS],
                     rhs=bg_row[:], start=True, stop=False)
    nc.tensor.matmul(out_ps[:], lhsT=hsT[:], rhs=WgT[:], start=False,
                     stop=True)
    out_sb = pd.tile([2 * T_STEPS, NCLS + 1], F32, name="out_sb")
    nc.vector.tensor_copy(out_sb[:], out_ps[:])
    nc.sync.dma_start(g["out_d"], out_sb[:, 0:NCLS])

    P.close("st_pool"); P.close("e_pool")


# ------------------------------------------------------------------ runner
_cache = {}


def kernel(image, text, conv_params, bn_params, dec_params):
    in_maps = _host_prep(image, text, conv_params, bn_params, dec_params)
    key = "k"
    if key not in _cache:
        _cache[key] = build_kernel(debug=bool(os.environ.get("KDEBUG")))
    nc = _cache[key]
    res = run_bass_kernel_spmd(nc, in_maps, core_ids=list(range(N_CORES)))
    out = np.concatenate(
        [res.results[c]["out"].reshape(BPC, T_STEPS, NCLS)
         for c in range(N_CORES)], axis=0)
    return out
